# revision 7
# baseline (speedup 1.0000x reference)
"""Trainium2 Bass kernel for nn_GCNDiehlq1 (SAGEConv x2 + lin + EdgePooling, x3
levels, mean readout + MLP + log_softmax).

Structure (validated against the jax reference at ~1e-7 rel err):
- All edges are within-graph, so SAGE mean aggregation == per-graph dense
  A_norm @ x with AT[s,d] = count(s->d)/max(indeg(d),1). The 64 graphs are
  sharded 8-per-core across 8 NeuronCores.
- The device kernel computes one level (2 SAGE convs + lin + edge-score
  projections u,v) for 8 graphs in feature-major layout. The builder is
  parameterized by slots-per-graph (320 for level 1; levels 2/3 pick
  256/128 variants based on actual post-pool graph sizes) -> one NEFF per
  size variant, reused across calls.
- Host does the inherently sequential EdgePooling (per-dst softmax from
  u,v, stable sort, greedy merge scan, coalesce) and the final readout.
  Per-graph compact cluster relabeling is strictly order-preserving
  w.r.t. the reference's global labels within each graph, so coalesce
  order and sort tie-breaks match the reference exactly.
"""

import numpy as np

N = 20480
NPG = 320
G = 64
F = 128
H = 128
C = 6
PADMAX = 384                 # host-side cluster-id padding (>= 320)
NCORES = 8
GPC = G // NCORES            # 8 graphs per core
NCHUNK = 512                 # dense matmul moving free dim

_compiled = {}


def _ktiles(sg):
    """K-tile (offset, length) list for sg slots per graph."""
    out = []
    off = 0
    while off < sg:
        out.append((off, min(128, sg - off)))
        off += 128
    return out


# ---------------------------------------------------------------- device ---

def _apply_tile_patch():
    """This walrus build rejects >1 sem waits on TPB_CTRL (Drain/NoOp):
    'Too many sync wait commands'. Split the TileContext exit-barrier waits
    across one NOP per logical proc, then emit the drain bare."""
    import concourse.tile as tile
    from concourse.vector_clock import ScopedClock, VectorClock

    if getattr(tile.TileContext, "_drain_patched", False):
        return

    def _patched(self, tick_clock, wait_clock):
        full = tick_clock.global_clock
        nprocs = len(full)
        for proc in range(nprocs):
            tick = full[proc]
            if tick <= 0:
                continue
            vec = [0] * nprocs
            vec[proc] = tick
            nop_inst = self.nc.sync.nop(nofuse=True, hint="pre_drain_%d" % proc)
            wait_clock.add_sem_waits(
                nop_inst.ins, ScopedClock({None: VectorClock(vec)})
            )
        self.nc.sync.drain()
        self.nc.all_engine_barrier()
        assert self.sems is not None
        popped = self.nc._tile_sem_poison_stack.pop()
        assert popped is self._sem_poison
        self.nc.clear_and_free_semaphores(list(self.sems.allocated().values()))
        self.nc.all_engine_barrier()

    tile.TileContext._drain_and_barrier = _patched
    tile.TileContext._drain_patched = True


def _split_multi_waits(nc):
    """This walrus build allows at most ONE sync wait per instruction.
    Insert single-wait NoOps (same engine, just before) for the extras."""
    import concourse.mybir as mybir

    for f in nc.m.functions:
        for bb in f.blocks:
            insts = list(bb.instructions)
            out = []
            changed = False
            for ins in insts:
                si = ins.sync_info
                if si is not None and len(si.on_wait) > 1:
                    waits = list(si.on_wait)
                    for j, w in enumerate(waits[:-1]):
                        nop = mybir.InstNoOp(name="%s_w%d" % (ins.name, j))
                        nop.engine = ins.engine
                        nop.sync_info = mybir.SyncInfo(on_wait=[w], on_update=[])
                        out.append(nop)
                    ins.sync_info = mybir.SyncInfo(
                        on_wait=[waits[-1]], on_update=list(si.on_update)
                    )
                    changed = True
                out.append(ins)
            if changed:
                bb.instructions = out


def _build_level_nc(sg):
    """One level for 8 graphs with sg slots per graph, feature-major
    activations [128, 8*sg]."""
    import concourse.bass as bass
    import concourse.mybir as mybir
    import concourse.tile as tile
    from concourse.masks import make_identity

    _apply_tile_patch()
    f32 = mybir.dt.float32
    AF = mybir.ActivationFunctionType

    kts = _ktiles(sg)
    nk = len(kts)
    npc = GPC * sg
    nch = (npc + NCHUNK - 1) // NCHUNK
    assert npc % NCHUNK == 0

    nc = bass.Bass("TRN2", target_bir_lowering=False)
    xT_d = nc.declare_dram_parameter("xT", [128, npc], f32, isOutput=False)
    xN_d = nc.declare_dram_parameter("xN", [128, GPC * nk * 128], f32,
                                     isOutput=False)
    at_d = nc.declare_dram_parameter("AT", [GPC * nk, 128, sg], f32,
                                     isOutput=False)
    w_d = {}
    for nm in ("WL1", "WR1", "WL2", "WR2", "WLA", "WLB"):
        w_d[nm] = nc.declare_dram_parameter(nm, [128, 128], f32, isOutput=False)
    w12_d = nc.declare_dram_parameter("w12", [128, 2], f32, isOutput=False)
    b_d = {}
    for nm in ("b1", "b2", "b3"):
        b_d[nm] = nc.declare_dram_parameter(nm, [128, 1], f32, isOutput=False)
    hout_d = nc.declare_dram_parameter("houtT", [128, npc], f32, isOutput=True)
    uv_d = nc.declare_dram_parameter("uv", [2, npc], f32, isOutput=True)

    with tile.TileContext(nc) as tc:
        with (
            tc.tile_pool(name="slab", bufs=1) as slab,
            tc.tile_pool(name="wts", bufs=1) as wts,
            tc.tile_pool(name="ps_agg", bufs=2, space="PSUM") as ps_agg,
            tc.tile_pool(name="ps_d", bufs=2, space="PSUM") as ps_d,
            tc.tile_pool(name="ps_tp", bufs=2, space="PSUM") as ps_tp,
            tc.tile_pool(name="ps_uv", bufs=2, space="PSUM") as ps_uv,
        ):
            ident = wts.tile([128, 128], f32)
            make_identity(nc, ident[:])

            W = {nm: wts.tile([128, 128], f32, name="W_" + nm, tag="W_" + nm)
                 for nm in w_d}
            for nm, d in w_d.items():
                nc.sync.dma_start(W[nm][:], d[:])
            w12 = wts.tile([128, 2], f32)
            nc.sync.dma_start(w12[:], w12_d[:])
            B = {nm: wts.tile([128, 1], f32, name="B_" + nm, tag="B_" + nm)
                 for nm in b_d}
            for nm, d in b_d.items():
                nc.sync.dma_start(B[nm][:], d[:])

            # split input DMAs for pipelining: per-graph AT + xN, per-chunk xT
            at = slab.tile([128, GPC * nk, sg], f32, tag="at")
            xN = slab.tile([128, GPC * nk * 128], f32, tag="xN")
            for g in range(GPC):
                for k in range(nk):
                    t = g * nk + k
                    nc.sync.dma_start(at[:, t, :], at_d[t])
                nc.sync.dma_start(
                    xN[:, g * nk * 128:(g + 1) * nk * 128],
                    xN_d[:, g * nk * 128:(g + 1) * nk * 128],
                )
            xT = slab.tile([128, npc], f32, tag="xT")
            for c in range(nch):
                sl = slice(c * NCHUNK, (c + 1) * NCHUNK)
                nc.sync.dma_start(xT[:, sl], xT_d[:, sl])

            def aggregate(nodemajor, name):
                """aggT[f, d] = sum_s x[s,f] * AT[s,d], per graph."""
                aggT = slab.tile([128, npc], f32, name=name, tag=name)
                for g in range(GPC):
                    ps = ps_agg.tile([128, sg], f32)
                    for k, (off, klen) in enumerate(kts):
                        t = g * nk + k
                        nc.tensor.matmul(
                            ps[:],
                            nodemajor[:klen, t * 128:t * 128 + 128],
                            at[:klen, t, :],
                            start=(k == 0),
                            stop=(k == nk - 1),
                        )
                    nc.vector.tensor_copy(aggT[:, g * sg:(g + 1) * sg], ps[:])
                return aggT

            def dense2(wa, rhsa, wb, rhsb, bias, func, name):
                """out[f',n] = func(wa.T@rhsa + wb.T@rhsb + bias)."""
                out = slab.tile([128, npc], f32, name=name, tag=name)
                for c in range(nch):
                    sl = slice(c * NCHUNK, (c + 1) * NCHUNK)
                    ps = ps_d.tile([128, NCHUNK], f32)
                    nc.tensor.matmul(ps[:], wa[:], rhsa[:, sl], start=True,
                                     stop=False)
                    nc.tensor.matmul(ps[:], wb[:], rhsb[:, sl], start=False,
                                     stop=True)
                    nc.scalar.activation(out[:, sl], ps[:], func, bias=bias[:])
                return out

            def to_nodemajor(featmajor, name):
                """Per-graph k-tiles: col block g*nk+k holds nodes
                [g*sg+off, g*sg+off+klen) in partitions [0, klen)."""
                out = slab.tile([128, GPC * nk * 128], f32, name=name, tag=name)
                for g in range(GPC):
                    for k, (off, klen) in enumerate(kts):
                        t = g * nk + k
                        ps = ps_tp.tile([128, 128], f32)
                        nc.tensor.transpose(
                            ps[:klen, :],
                            featmajor[:, g * sg + off:g * sg + off + klen],
                            ident[:],
                        )
                        nc.vector.tensor_copy(
                            out[:klen, t * 128:t * 128 + 128], ps[:klen, :]
                        )
                return out

            agg1 = aggregate(xN, "agg1")
            h1 = dense2(W["WL1"], agg1, W["WR1"], xT, B["b1"], AF.Relu, "h1")
            h1N = to_nodemajor(h1, "h1N")
            agg2 = aggregate(h1N, "agg2")
            h2 = dense2(W["WL2"], agg2, W["WR2"], h1, B["b2"], AF.Relu, "h2")
            hout = dense2(W["WLA"], h2, W["WLB"], h1, B["b3"], AF.Identity,
                          "hout")

            uvT = slab.tile([2, npc], f32, tag="uv")
            for c in range(nch):
                sl = slice(c * NCHUNK, (c + 1) * NCHUNK)
                ps = ps_uv.tile([2, NCHUNK], f32)
                nc.tensor.matmul(ps[:], w12[:], hout[:, sl], start=True,
                                 stop=True)
                nc.vector.tensor_copy(uvT[:, sl], ps[:])
                # split output DMAs per chunk so stores overlap compute
                nc.sync.dma_start(hout_d[:, sl], hout[:, sl])
                nc.sync.dma_start(uv_d[:, sl], uvT[:, sl])

    _split_multi_waits(nc)
    return nc


def _get_level_nc(sg):
    if sg not in _compiled:
        _compiled[sg] = _build_level_nc(sg)
    return _compiled[sg]


def _run_level(Xslots, AThost, W, n, sg):
    """Xslots: [G, PADMAX, F] node-major; AThost: [G, sg, sg] normalized.
    W: level weight dict; n: [G] active counts (n <= sg).
    Returns Hout [G, sg, F], U [G, sg], V [G, sg] (all fp32)."""
    from concourse.bass_utils import run_bass_kernel_spmd

    nc = _get_level_nc(sg)
    kts = _ktiles(sg)
    nk = len(kts)
    npc = GPC * sg
    shared = {
        "WL1": np.ascontiguousarray(W["Wl1"].T),
        "WR1": np.ascontiguousarray(W["Wr1"].T),
        "WL2": np.ascontiguousarray(W["Wl2"].T),
        "WR2": np.ascontiguousarray(W["Wr2"].T),
        "WLA": np.ascontiguousarray(W["WlinA"]),
        "WLB": np.ascontiguousarray(W["WlinB"]),
        "w12": np.ascontiguousarray(np.stack([W["w1"], W["w2"]], axis=1)),
        "b1": np.ascontiguousarray(W["bl1"].reshape(128, 1)),
        "b2": np.ascontiguousarray(W["bl2"].reshape(128, 1)),
        "b3": np.ascontiguousarray(W["blin"].reshape(128, 1)),
    }
    in_maps = []
    for c in range(NCORES):
        xs = Xslots[c * GPC:(c + 1) * GPC, :sg].reshape(npc, F)
        xT = np.ascontiguousarray(xs.T)
        xN = np.zeros((128, GPC * nk * 128), np.float32)
        at = np.zeros((GPC * nk, 128, sg), np.float32)
        for g in range(GPC):
            for k, (off, klen) in enumerate(kts):
                t = g * nk + k
                blk = Xslots[c * GPC + g, off:off + klen]        # [klen, F]
                xN[:klen, t * 128:t * 128 + 128] = blk
                at[t, :klen, :] = AThost[c * GPC + g, off:off + klen, :]
        in_maps.append({"xT": xT, "xN": xN, "AT": at, **shared})

    res = run_bass_kernel_spmd(nc, in_maps, list(range(NCORES)))
    Hout = np.empty((G, sg, F), np.float32)
    U = np.empty((G, sg), np.float32)
    V = np.empty((G, sg), np.float32)
    for c in range(NCORES):
        h = res.results[c]["houtT"]          # [128, npc]
        uv = res.results[c]["uv"]            # [2, npc]
        Hout[c * GPC:(c + 1) * GPC] = h.T.reshape(GPC, sg, F)
        U[c * GPC:(c + 1) * GPC] = uv[0].reshape(GPC, sg)
        V[c * GPC:(c + 1) * GPC] = uv[1].reshape(GPC, sg)
    return Hout, U, V


# ------------------------------------------------------------------ host ---

def _build_AT(edges, sg):
    AT = np.zeros((G, sg, sg), np.float32)
    for g in range(G):
        ls, ld = edges[g]
        if len(ls) == 0:
            continue
        cnt = np.bincount(ls.astype(np.int64) * sg + ld, minlength=sg * sg)
        cnt = cnt.reshape(sg, sg).astype(np.float32)
        indeg = np.bincount(ld, minlength=sg).astype(np.float32)
        AT[g] = cnt / np.maximum(indeg, 1.0)[None, :]
    return AT


def _pool_graph(Hout_g, u_g, v_g, ls, ld, n_g, bp):
    raw = u_g[ls] + v_g[ld] + np.float32(bp)
    m = np.full(n_g, -np.inf, np.float32)
    np.maximum.at(m, ld, raw)
    e = np.exp(raw - m[ld], dtype=np.float32)
    z = np.bincount(ld, weights=e, minlength=n_g).astype(np.float32)
    score = e / z[ld] + np.float32(0.5)

    order = np.argsort(-score, kind="stable")
    rem = [True] * n_g
    cluster = np.empty(n_g, np.int64)
    cnt = 0
    cscores = []
    ls_l = ls.tolist()
    ld_l = ld.tolist()
    sc_l = score.tolist()
    for idx in order.tolist():
        s = ls_l[idx]
        d = ld_l[idx]
        if rem[s] and rem[d]:
            cluster[s] = cnt
            cluster[d] = cnt
            rem[s] = False
            rem[d] = False
            cscores.append(sc_l[idx])
            cnt += 1
    rem_nodes = np.flatnonzero(rem)
    cluster[rem_nodes] = cnt + np.arange(len(rem_nodes))
    n_new = cnt + len(rem_nodes)

    csc = np.concatenate(
        [np.asarray(cscores, np.float32), np.ones(len(rem_nodes), np.float32)]
    )
    newX = np.zeros((PADMAX, F), np.float32)
    np.add.at(newX, cluster, Hout_g[:n_g])
    newX[:n_new] *= csc[:, None]

    keys = np.unique(cluster[ls] * PADMAX + cluster[ld])
    return newX, n_new, (
        (keys // PADMAX).astype(np.int32),
        (keys % PADMAX).astype(np.int32),
    )


def _level_weights(params, i):
    Wlin = params["Wlin%d" % i]
    Wp = params["Wp%d" % i]
    return {
        "Wl1": params["W%dl" % (2 * i - 1)],
        "bl1": params["b%dl" % (2 * i - 1)],
        "Wr1": params["W%dr" % (2 * i - 1)],
        "Wl2": params["W%dl" % (2 * i)],
        "bl2": params["b%dl" % (2 * i)],
        "Wr2": params["W%dr" % (2 * i)],
        "WlinA": np.ascontiguousarray(Wlin[:, :H].T),
        "WlinB": np.ascontiguousarray(Wlin[:, H:].T),
        "blin": params["blin%d" % i],
        "w1": np.ascontiguousarray(Wp[0, :H]),
        "w2": np.ascontiguousarray(Wp[0, H:]),
        "bp": float(np.asarray(params["bp%d" % i]).reshape(-1)[0]),
    }


def _log_softmax(x):
    m = x.max(axis=1, keepdims=True)
    e = np.exp(x - m)
    return (x - m) - np.log(e.sum(axis=1, keepdims=True))


def _pick_sg(nmax):
    for sg in (128, 256, 320):
        if nmax <= sg:
            return sg
    raise AssertionError("graph size %d exceeds 320" % nmax)


def kernel(x, edge_index, batch, params):
    x = np.asarray(x, np.float32)
    params = {k: np.asarray(v, np.float32) for k, v in params.items()}
    src = np.asarray(edge_index[0], np.int64)
    dst = np.asarray(edge_index[1], np.int64)

    g_of_edge = src // NPG
    order = np.argsort(g_of_edge, kind="stable")
    bounds = np.searchsorted(g_of_edge[order], np.arange(G + 1))
    edges = []
    for g in range(G):
        sel = order[bounds[g]: bounds[g + 1]]
        edges.append(((src[sel] - g * NPG).astype(np.int32),
                      (dst[sel] - g * NPG).astype(np.int32)))

    X = np.zeros((G, PADMAX, F), np.float32)
    X[:, :NPG] = x.reshape(G, NPG, F)
    n = np.full(G, NPG, np.int64)

    total_sum = np.zeros((G, F), np.float32)
    total_cnt = np.zeros(G, np.int64)

    for i in (1, 2, 3):
        W = _level_weights(params, i)
        sg = _pick_sg(int(n.max()))
        AT = _build_AT(edges, sg)
        Hout, U, V = _run_level(X, AT, W, n, sg)
        newX = np.zeros((G, PADMAX, F), np.float32)
        new_n = np.empty(G, np.int64)
        new_edges = []
        for g in range(G):
            total_sum[g] += Hout[g, : n[g]].sum(axis=0)
            total_cnt[g] += n[g]
            nx, nn, ne = _pool_graph(
                Hout[g], U[g], V[g], edges[g][0], edges[g][1], n[g], W["bp"]
            )
            newX[g] = nx
            new_n[g] = nn
            new_edges.append(ne)
        X, n, edges = newX, new_n, new_edges

    for g in range(G):
        total_sum[g] += X[g, : n[g]].sum(axis=0)
        total_cnt[g] += n[g]

    gv = total_sum / np.maximum(total_cnt, 1)[:, None].astype(np.float32)
    g1 = np.maximum(gv @ params["Wfc1"].T + params["bfc1"], 0.0)
    out = g1 @ params["Wfc2"].T + params["bfc2"]
    return _log_softmax(out).astype(np.float32)


# revision 11
# speedup vs baseline: 1.3169x; 1.3169x over previous
"""Trainium2 Bass kernel for nn_GCNDiehlq1 (SAGEConv x2 + lin + EdgePooling, x3
levels, mean readout + MLP + log_softmax).

Structure (validated against the jax reference at ~1e-7 rel err):
- All edges are within-graph, so SAGE mean aggregation == per-graph dense
  A_norm @ x with AT[s,d] = count(s->d)/max(indeg(d),1). The 64 graphs are
  sharded 8-per-core across 8 NeuronCores.
- The device kernel computes one level (2 SAGE convs + lin + edge-score
  projections u,v) for 8 graphs in feature-major layout. The builder is
  parameterized by slots-per-graph (320 for level 1; levels 2/3 pick
  256/128 variants based on actual post-pool graph sizes) -> one NEFF per
  size variant, reused across calls.
- Host does the inherently sequential EdgePooling (per-dst softmax from
  u,v, stable sort, greedy merge scan, coalesce) and the final readout.
  Per-graph compact cluster relabeling is strictly order-preserving
  w.r.t. the reference's global labels within each graph, so coalesce
  order and sort tie-breaks match the reference exactly.
"""

import numpy as np

N = 20480
NPG = 320
G = 64
F = 128
H = 128
C = 6
PADMAX = 384                 # host-side cluster-id padding (>= 320)
NCORES = 8
GPC = G // NCORES            # 8 graphs per core
NCHUNK = 512                 # dense matmul moving free dim

_compiled = {}


def _ktiles(sg):
    """K-tile (offset, length) list for sg slots per graph."""
    out = []
    off = 0
    while off < sg:
        out.append((off, min(128, sg - off)))
        off += 128
    return out


# ---------------------------------------------------------------- device ---

def _apply_tile_patch():
    """This walrus build rejects >1 sem waits on TPB_CTRL (Drain/NoOp):
    'Too many sync wait commands'. Split the TileContext exit-barrier waits
    across one NOP per logical proc, then emit the drain bare."""
    import concourse.tile as tile
    from concourse.vector_clock import ScopedClock, VectorClock

    if getattr(tile.TileContext, "_drain_patched", False):
        return

    def _patched(self, tick_clock, wait_clock):
        full = tick_clock.global_clock
        nprocs = len(full)
        for proc in range(nprocs):
            tick = full[proc]
            if tick <= 0:
                continue
            vec = [0] * nprocs
            vec[proc] = tick
            nop_inst = self.nc.sync.nop(nofuse=True, hint="pre_drain_%d" % proc)
            wait_clock.add_sem_waits(
                nop_inst.ins, ScopedClock({None: VectorClock(vec)})
            )
        self.nc.sync.drain()
        self.nc.all_engine_barrier()
        assert self.sems is not None
        popped = self.nc._tile_sem_poison_stack.pop()
        assert popped is self._sem_poison
        self.nc.clear_and_free_semaphores(list(self.sems.allocated().values()))
        self.nc.all_engine_barrier()

    tile.TileContext._drain_and_barrier = _patched
    tile.TileContext._drain_patched = True


def _split_multi_waits(nc):
    """This walrus build allows at most ONE sync wait per instruction.
    Insert single-wait NoOps (same engine, just before) for the extras."""
    import concourse.mybir as mybir

    for f in nc.m.functions:
        for bb in f.blocks:
            insts = list(bb.instructions)
            out = []
            changed = False
            for ins in insts:
                si = ins.sync_info
                if si is not None and len(si.on_wait) > 1:
                    waits = list(si.on_wait)
                    for j, w in enumerate(waits[:-1]):
                        nop = mybir.InstNoOp(name="%s_w%d" % (ins.name, j))
                        nop.engine = ins.engine
                        nop.sync_info = mybir.SyncInfo(on_wait=[w], on_update=[])
                        out.append(nop)
                    ins.sync_info = mybir.SyncInfo(
                        on_wait=[waits[-1]], on_update=list(si.on_update)
                    )
                    changed = True
                out.append(ins)
            if changed:
                bb.instructions = out


def _build_level_nc(sg, use_bf16):
    """One level for 8 graphs with sg slots per graph, feature-major
    activations [128, 8*sg]."""
    import concourse.bass as bass
    import concourse.mybir as mybir
    import concourse.tile as tile
    from concourse.masks import make_identity

    _apply_tile_patch()
    f32 = mybir.dt.float32
    cdt = mybir.dt.bfloat16 if use_bf16 else f32
    AF = mybir.ActivationFunctionType

    kts = _ktiles(sg)
    nk = len(kts)
    npc = GPC * sg
    nch = (npc + NCHUNK - 1) // NCHUNK
    assert npc % NCHUNK == 0

    nc = bass.Bass("TRN2", target_bir_lowering=False)
    xT_d = nc.declare_dram_parameter("xT", [128, npc], cdt, isOutput=False)
    xN_d = nc.declare_dram_parameter("xN", [128, GPC * nk * 128], cdt,
                                     isOutput=False)
    at_d = nc.declare_dram_parameter("AT", [GPC * nk, 128, sg], cdt,
                                     isOutput=False)
    w_d = {}
    for nm in ("WL1", "WR1", "WL2", "WR2", "WLA", "WLB"):
        w_d[nm] = nc.declare_dram_parameter(nm, [128, 128], cdt, isOutput=False)
    w12_d = nc.declare_dram_parameter("w12", [128, 2], cdt, isOutput=False)
    b_d = {}
    for nm in ("b1", "b2", "b3"):
        b_d[nm] = nc.declare_dram_parameter(nm, [128, 1], f32, isOutput=False)
    hout_d = nc.declare_dram_parameter("houtT", [128, npc], cdt, isOutput=True)
    uv_d = nc.declare_dram_parameter("uv", [2, npc], f32, isOutput=True)

    with tile.TileContext(nc) as tc:
        with (
            tc.tile_pool(name="slab", bufs=1) as slab,
            tc.tile_pool(name="wts", bufs=1) as wts,
            tc.tile_pool(name="ps_agg", bufs=2, space="PSUM") as ps_agg,
            tc.tile_pool(name="ps_d", bufs=2, space="PSUM") as ps_d,
            tc.tile_pool(name="ps_tp", bufs=2, space="PSUM") as ps_tp,
            tc.tile_pool(name="ps_uv", bufs=2, space="PSUM") as ps_uv,
        ):
            ident = wts.tile([128, 128], cdt)
            make_identity(nc, ident[:])

            W = {nm: wts.tile([128, 128], cdt, name="W_" + nm, tag="W_" + nm)
                 for nm in w_d}
            for nm, d in w_d.items():
                nc.sync.dma_start(W[nm][:], d[:])
            w12 = wts.tile([128, 2], cdt)
            nc.sync.dma_start(w12[:], w12_d[:])
            B = {nm: wts.tile([128, 1], f32, name="B_" + nm, tag="B_" + nm)
                 for nm in b_d}
            for nm, d in b_d.items():
                nc.sync.dma_start(B[nm][:], d[:])

            # split input DMAs for pipelining: per-graph AT + xN, per-chunk xT
            at = slab.tile([128, GPC * nk, sg], cdt, tag="at")
            xN = slab.tile([128, GPC * nk * 128], cdt, tag="xN")
            for g in range(GPC):
                for k in range(nk):
                    t = g * nk + k
                    nc.sync.dma_start(at[:, t, :], at_d[t])
                nc.sync.dma_start(
                    xN[:, g * nk * 128:(g + 1) * nk * 128],
                    xN_d[:, g * nk * 128:(g + 1) * nk * 128],
                )
            xT = slab.tile([128, npc], cdt, tag="xT")
            for c in range(nch):
                sl = slice(c * NCHUNK, (c + 1) * NCHUNK)
                nc.sync.dma_start(xT[:, sl], xT_d[:, sl])

            def aggregate(nodemajor, name):
                """aggT[f, d] = sum_s x[s,f] * AT[s,d], per graph."""
                aggT = slab.tile([128, npc], cdt, name=name, tag=name)
                for g in range(GPC):
                    ps = ps_agg.tile([128, sg], f32)
                    for k, (off, klen) in enumerate(kts):
                        t = g * nk + k
                        nc.tensor.matmul(
                            ps[:],
                            nodemajor[:klen, t * 128:t * 128 + 128],
                            at[:klen, t, :],
                            start=(k == 0),
                            stop=(k == nk - 1),
                        )
                    nc.vector.tensor_copy(aggT[:, g * sg:(g + 1) * sg], ps[:])
                return aggT

            def dense2(wa, rhsa, wb, rhsb, bias, func, name):
                """out[f',n] = func(wa.T@rhsa + wb.T@rhsb + bias)."""
                out = slab.tile([128, npc], cdt, name=name, tag=name)
                for c in range(nch):
                    sl = slice(c * NCHUNK, (c + 1) * NCHUNK)
                    ps = ps_d.tile([128, NCHUNK], f32)
                    nc.tensor.matmul(ps[:], wa[:], rhsa[:, sl], start=True,
                                     stop=False)
                    nc.tensor.matmul(ps[:], wb[:], rhsb[:, sl], start=False,
                                     stop=True)
                    nc.scalar.activation(out[:, sl], ps[:], func, bias=bias[:])
                return out

            def to_nodemajor(featmajor, name):
                """Per-graph k-tiles: col block g*nk+k holds nodes
                [g*sg+off, g*sg+off+klen) in partitions [0, klen)."""
                out = slab.tile([128, GPC * nk * 128], cdt, name=name, tag=name)
                for g in range(GPC):
                    for k, (off, klen) in enumerate(kts):
                        t = g * nk + k
                        ps = ps_tp.tile([128, 128], cdt)
                        nc.tensor.transpose(
                            ps[:klen, :],
                            featmajor[:, g * sg + off:g * sg + off + klen],
                            ident[:],
                        )
                        nc.vector.tensor_copy(
                            out[:klen, t * 128:t * 128 + 128], ps[:klen, :]
                        )
                return out

            agg1 = aggregate(xN, "agg1")
            h1 = dense2(W["WL1"], agg1, W["WR1"], xT, B["b1"], AF.Relu, "h1")
            h1N = to_nodemajor(h1, "h1N")
            agg2 = aggregate(h1N, "agg2")
            h2 = dense2(W["WL2"], agg2, W["WR2"], h1, B["b2"], AF.Relu, "h2")
            hout = dense2(W["WLA"], h2, W["WLB"], h1, B["b3"], AF.Identity,
                          "hout")

            uvT = slab.tile([2, npc], f32, tag="uv")
            for c in range(nch):
                sl = slice(c * NCHUNK, (c + 1) * NCHUNK)
                ps = ps_uv.tile([2, NCHUNK], f32)
                nc.tensor.matmul(ps[:], w12[:], hout[:, sl], start=True,
                                 stop=True)
                nc.vector.tensor_copy(uvT[:, sl], ps[:])
                # split output DMAs per chunk so stores overlap compute
                nc.sync.dma_start(hout_d[:, sl], hout[:, sl])
                nc.sync.dma_start(uv_d[:, sl], uvT[:, sl])

    _split_multi_waits(nc)
    return nc


USE_BF16 = True


def _np_cdt():
    if USE_BF16:
        import ml_dtypes

        return ml_dtypes.bfloat16
    return np.float32


def _get_level_nc(sg):
    key = (sg, USE_BF16)
    if key not in _compiled:
        _compiled[key] = _build_level_nc(sg, USE_BF16)
    return _compiled[key]


def _run_level(Xslots, AThost, W, n, sg):
    """Xslots: [G, PADMAX, F] node-major; AThost: [G, sg, sg] normalized.
    W: level weight dict; n: [G] active counts (n <= sg).
    Returns Hout [G, sg, F], U [G, sg], V [G, sg] (all fp32)."""
    from concourse.bass_utils import run_bass_kernel_spmd

    nc = _get_level_nc(sg)
    kts = _ktiles(sg)
    nk = len(kts)
    npc = GPC * sg
    cdt = _np_cdt()
    shared = {
        "WL1": np.ascontiguousarray(W["Wl1"].T.astype(cdt)),
        "WR1": np.ascontiguousarray(W["Wr1"].T.astype(cdt)),
        "WL2": np.ascontiguousarray(W["Wl2"].T.astype(cdt)),
        "WR2": np.ascontiguousarray(W["Wr2"].T.astype(cdt)),
        "WLA": np.ascontiguousarray(W["WlinA"].astype(cdt)),
        "WLB": np.ascontiguousarray(W["WlinB"].astype(cdt)),
        "w12": np.ascontiguousarray(
            np.stack([W["w1"], W["w2"]], axis=1).astype(cdt)),
        "b1": np.ascontiguousarray(W["bl1"].reshape(128, 1)),
        "b2": np.ascontiguousarray(W["bl2"].reshape(128, 1)),
        "b3": np.ascontiguousarray(W["blin"].reshape(128, 1)),
    }
    in_maps = []
    for c in range(NCORES):
        xs = Xslots[c * GPC:(c + 1) * GPC, :sg].reshape(npc, F).astype(cdt)
        xT = np.ascontiguousarray(xs.T)
        xN = np.zeros((128, GPC * nk * 128), cdt)
        at = np.zeros((GPC * nk, 128, sg), cdt)
        for g in range(GPC):
            for k, (off, klen) in enumerate(kts):
                t = g * nk + k
                blk = xs.reshape(GPC, sg, F)[g, off:off + klen]  # [klen, F]
                xN[:klen, t * 128:t * 128 + 128] = blk
                at[t, :klen, :] = AThost[c * GPC + g, off:off + klen, :].astype(cdt)
        in_maps.append({"xT": xT, "xN": xN, "AT": at, **shared})

    res = run_bass_kernel_spmd(nc, in_maps, list(range(NCORES)))
    Hout = np.empty((G, sg, F), np.float32)
    U = np.empty((G, sg), np.float32)
    V = np.empty((G, sg), np.float32)
    for c in range(NCORES):
        h = res.results[c]["houtT"].astype(np.float32)   # [128, npc]
        uv = res.results[c]["uv"]                        # [2, npc]
        Hout[c * GPC:(c + 1) * GPC] = h.T.reshape(GPC, sg, F)
        U[c * GPC:(c + 1) * GPC] = uv[0].reshape(GPC, sg)
        V[c * GPC:(c + 1) * GPC] = uv[1].reshape(GPC, sg)
    return Hout, U, V


# ------------------------------------------------------------------ host ---

def _build_AT(edges, sg):
    AT = np.zeros((G, sg, sg), np.float32)
    for g in range(G):
        ls, ld = edges[g]
        if len(ls) == 0:
            continue
        cnt = np.bincount(ls.astype(np.int64) * sg + ld, minlength=sg * sg)
        cnt = cnt.reshape(sg, sg).astype(np.float32)
        indeg = np.bincount(ld, minlength=sg).astype(np.float32)
        AT[g] = cnt / np.maximum(indeg, 1.0)[None, :]
    return AT


def _pool_graph(Hout_g, u_g, v_g, ls, ld, n_g, bp):
    raw = u_g[ls] + v_g[ld] + np.float32(bp)
    m = np.full(n_g, -np.inf, np.float32)
    np.maximum.at(m, ld, raw)
    e = np.exp(raw - m[ld], dtype=np.float32)
    z = np.bincount(ld, weights=e, minlength=n_g).astype(np.float32)
    score = e / z[ld] + np.float32(0.5)

    order = np.argsort(-score, kind="stable")
    rem = [True] * n_g
    cluster = np.empty(n_g, np.int64)
    cnt = 0
    cscores = []
    ls_l = ls.tolist()
    ld_l = ld.tolist()
    sc_l = score.tolist()
    for idx in order.tolist():
        s = ls_l[idx]
        d = ld_l[idx]
        if rem[s] and rem[d]:
            cluster[s] = cnt
            cluster[d] = cnt
            rem[s] = False
            rem[d] = False
            cscores.append(sc_l[idx])
            cnt += 1
    rem_nodes = np.flatnonzero(rem)
    cluster[rem_nodes] = cnt + np.arange(len(rem_nodes))
    n_new = cnt + len(rem_nodes)

    csc = np.concatenate(
        [np.asarray(cscores, np.float32), np.ones(len(rem_nodes), np.float32)]
    )
    newX = np.zeros((PADMAX, F), np.float32)
    np.add.at(newX, cluster, Hout_g[:n_g])
    newX[:n_new] *= csc[:, None]

    keys = np.unique(cluster[ls] * PADMAX + cluster[ld])
    return newX, n_new, (
        (keys // PADMAX).astype(np.int32),
        (keys % PADMAX).astype(np.int32),
    )


def _level_weights(params, i):
    Wlin = params["Wlin%d" % i]
    Wp = params["Wp%d" % i]
    return {
        "Wl1": params["W%dl" % (2 * i - 1)],
        "bl1": params["b%dl" % (2 * i - 1)],
        "Wr1": params["W%dr" % (2 * i - 1)],
        "Wl2": params["W%dl" % (2 * i)],
        "bl2": params["b%dl" % (2 * i)],
        "Wr2": params["W%dr" % (2 * i)],
        "WlinA": np.ascontiguousarray(Wlin[:, :H].T),
        "WlinB": np.ascontiguousarray(Wlin[:, H:].T),
        "blin": params["blin%d" % i],
        "w1": np.ascontiguousarray(Wp[0, :H]),
        "w2": np.ascontiguousarray(Wp[0, H:]),
        "bp": float(np.asarray(params["bp%d" % i]).reshape(-1)[0]),
    }


def _log_softmax(x):
    m = x.max(axis=1, keepdims=True)
    e = np.exp(x - m)
    return (x - m) - np.log(e.sum(axis=1, keepdims=True))


def _pick_sg(nmax):
    for sg in (128, 256, 320):
        if nmax <= sg:
            return sg
    raise AssertionError("graph size %d exceeds 320" % nmax)


def kernel(x, edge_index, batch, params):
    x = np.asarray(x, np.float32)
    params = {k: np.asarray(v, np.float32) for k, v in params.items()}
    src = np.asarray(edge_index[0], np.int64)
    dst = np.asarray(edge_index[1], np.int64)

    g_of_edge = src // NPG
    order = np.argsort(g_of_edge, kind="stable")
    bounds = np.searchsorted(g_of_edge[order], np.arange(G + 1))
    edges = []
    for g in range(G):
        sel = order[bounds[g]: bounds[g + 1]]
        edges.append(((src[sel] - g * NPG).astype(np.int32),
                      (dst[sel] - g * NPG).astype(np.int32)))

    X = np.zeros((G, PADMAX, F), np.float32)
    X[:, :NPG] = x.reshape(G, NPG, F)
    n = np.full(G, NPG, np.int64)

    total_sum = np.zeros((G, F), np.float32)
    total_cnt = np.zeros(G, np.int64)

    for i in (1, 2, 3):
        W = _level_weights(params, i)
        sg = _pick_sg(int(n.max()))
        AT = _build_AT(edges, sg)
        Hout, U, V = _run_level(X, AT, W, n, sg)
        newX = np.zeros((G, PADMAX, F), np.float32)
        new_n = np.empty(G, np.int64)
        new_edges = []
        for g in range(G):
            total_sum[g] += Hout[g, : n[g]].sum(axis=0)
            total_cnt[g] += n[g]
            nx, nn, ne = _pool_graph(
                Hout[g], U[g], V[g], edges[g][0], edges[g][1], n[g], W["bp"]
            )
            newX[g] = nx
            new_n[g] = nn
            new_edges.append(ne)
        X, n, edges = newX, new_n, new_edges

    for g in range(G):
        total_sum[g] += X[g, : n[g]].sum(axis=0)
        total_cnt[g] += n[g]

    gv = total_sum / np.maximum(total_cnt, 1)[:, None].astype(np.float32)
    g1 = np.maximum(gv @ params["Wfc1"].T + params["bfc1"], 0.0)
    out = g1 @ params["Wfc2"].T + params["bfc2"]
    return _log_softmax(out).astype(np.float32)


# revision 18
# speedup vs baseline: 1.8093x; 1.3739x over previous
"""Trainium2 Bass kernel for nn_GCNDiehlq1 (SAGEConv x2 + lin + EdgePooling, x3
levels, mean readout + MLP + log_softmax).

Structure (validated against the jax reference at ~1e-7 rel err):
- All edges are within-graph, so SAGE mean aggregation == per-graph dense
  A_norm @ x with AT[s,d] = count(s->d)/max(indeg(d),1). The 64 graphs are
  sharded 8-per-core across 8 NeuronCores.
- The device kernel computes one level (2 SAGE convs + lin + edge-score
  projections u,v) for 8 graphs in feature-major layout. The builder is
  parameterized by slots-per-graph (320 for level 1; levels 2/3 pick
  256/128 variants based on actual post-pool graph sizes) -> one NEFF per
  size variant, reused across calls.
- Host does the inherently sequential EdgePooling (per-dst softmax from
  u,v, stable sort, greedy merge scan, coalesce) and the final readout.
  Per-graph compact cluster relabeling is strictly order-preserving
  w.r.t. the reference's global labels within each graph, so coalesce
  order and sort tie-breaks match the reference exactly.
"""

import numpy as np

N = 20480
NPG = 320
G = 64
F = 128
H = 128
C = 6
PADMAX = 384                 # host-side cluster-id padding (>= 320)
NCORES = 8
GPC = G // NCORES            # 8 graphs per core
NCHUNK = 512                 # dense matmul moving free dim

_compiled = {}


def _ktiles(sg):
    """K-tile (offset, length) list for sg slots per graph."""
    out = []
    off = 0
    while off < sg:
        out.append((off, min(128, sg - off)))
        off += 128
    return out


# ---------------------------------------------------------------- device ---

def _apply_tile_patch():
    """This walrus build rejects >1 sem waits on TPB_CTRL (Drain/NoOp):
    'Too many sync wait commands'. Split the TileContext exit-barrier waits
    across one NOP per logical proc, then emit the drain bare."""
    import concourse.tile as tile
    from concourse.vector_clock import ScopedClock, VectorClock

    if getattr(tile.TileContext, "_drain_patched", False):
        return

    def _patched(self, tick_clock, wait_clock):
        full = tick_clock.global_clock
        nprocs = len(full)
        for proc in range(nprocs):
            tick = full[proc]
            if tick <= 0:
                continue
            vec = [0] * nprocs
            vec[proc] = tick
            nop_inst = self.nc.sync.nop(nofuse=True, hint="pre_drain_%d" % proc)
            wait_clock.add_sem_waits(
                nop_inst.ins, ScopedClock({None: VectorClock(vec)})
            )
        self.nc.sync.drain()
        self.nc.all_engine_barrier()
        assert self.sems is not None
        popped = self.nc._tile_sem_poison_stack.pop()
        assert popped is self._sem_poison
        self.nc.clear_and_free_semaphores(list(self.sems.allocated().values()))
        self.nc.all_engine_barrier()

    tile.TileContext._drain_and_barrier = _patched
    tile.TileContext._drain_patched = True


def _split_multi_waits(nc):
    """This walrus build allows at most ONE sync wait per instruction.
    Insert single-wait NoOps (same engine, just before) for the extras."""
    import concourse.mybir as mybir

    for f in nc.m.functions:
        for bb in f.blocks:
            insts = list(bb.instructions)
            out = []
            changed = False
            for ins in insts:
                si = ins.sync_info
                if si is not None and len(si.on_wait) > 1:
                    waits = list(si.on_wait)
                    for j, w in enumerate(waits[:-1]):
                        nop = mybir.InstNoOp(name="%s_w%d" % (ins.name, j))
                        nop.engine = ins.engine
                        nop.sync_info = mybir.SyncInfo(on_wait=[w], on_update=[])
                        out.append(nop)
                    ins.sync_info = mybir.SyncInfo(
                        on_wait=[waits[-1]], on_update=list(si.on_update)
                    )
                    changed = True
                out.append(ins)
            if changed:
                bb.instructions = out


def _build_level_nc(sg, use_bf16):
    """One level for 8 graphs with sg slots per graph, feature-major
    activations [128, 8*sg]."""
    import concourse.bass as bass
    import concourse.mybir as mybir
    import concourse.tile as tile
    from concourse.masks import make_identity

    _apply_tile_patch()
    f32 = mybir.dt.float32
    cdt = mybir.dt.bfloat16 if use_bf16 else f32
    AF = mybir.ActivationFunctionType

    kts = _ktiles(sg)
    nk = len(kts)
    npc = GPC * sg
    nch = (npc + NCHUNK - 1) // NCHUNK
    assert npc % NCHUNK == 0

    nc = bass.Bass("TRN2", target_bir_lowering=False)
    xT_d = nc.declare_dram_parameter("xT", [128, npc], cdt, isOutput=False)
    at_d = nc.declare_dram_parameter("AT", [128, GPC * nk, sg], cdt,
                                     isOutput=False)
    # all six [128,128] weights + w12 packed into one [128, 770] slab
    wpack_d = nc.declare_dram_parameter("wpack", [128, 6 * 128 + 2], cdt,
                                        isOutput=False)
    bpack_d = nc.declare_dram_parameter("bpack", [128, 3], f32, isOutput=False)
    hout_d = nc.declare_dram_parameter("houtT", [128, npc], cdt, isOutput=True)
    uv_d = nc.declare_dram_parameter("uv", [2, npc], f32, isOutput=True)

    with tile.TileContext(nc) as tc:
        with (
            tc.tile_pool(name="slab", bufs=1) as slab,
            tc.tile_pool(name="wts", bufs=1) as wts,
            tc.tile_pool(name="ps_agg", bufs=2, space="PSUM") as ps_agg,
            tc.tile_pool(name="ps_d", bufs=2, space="PSUM") as ps_d,
            tc.tile_pool(name="ps_tp", bufs=2, space="PSUM") as ps_tp,
            tc.tile_pool(name="ps_uv", bufs=2, space="PSUM") as ps_uv,
        ):
            ident = wts.tile([128, 128], cdt)
            make_identity(nc, ident[:])

            # weights via gpsimd (SWDGE) to keep the SP queue free for inputs
            wpack = wts.tile([128, 6 * 128 + 2], cdt)
            nc.gpsimd.dma_start(wpack[:], wpack_d[:])
            bpack = wts.tile([128, 3], f32)
            nc.gpsimd.dma_start(bpack[:], bpack_d[:])
            wnames = ("WL1", "WR1", "WL2", "WR2", "WLA", "WLB")
            W = {nm: wpack[:, i * 128:(i + 1) * 128]
                 for i, nm in enumerate(wnames)}
            w12 = wpack[:, 6 * 128:6 * 128 + 2]
            B = {"b%d" % (i + 1): bpack[:, i:i + 1] for i in range(3)}

            # inputs on the SP queue: per-chunk xT first, then per-graph AT
            xT = slab.tile([128, npc], cdt, tag="xT")
            for c in range(nch):
                sl = slice(c * NCHUNK, (c + 1) * NCHUNK)
                nc.sync.dma_start(xT[:, sl], xT_d[:, sl])
            at = slab.tile([128, GPC * nk, sg], cdt, tag="at")
            for g in range(GPC):
                gsl = slice(g * nk, (g + 1) * nk)
                nc.sync.dma_start(at[:, gsl, :], at_d[:, gsl, :])

            def aggregate(nodemajor, name):
                """aggT[f, d] = sum_s x[s,f] * AT[s,d], per graph."""
                aggT = slab.tile([128, npc], cdt, name=name, tag=name)
                for g in range(GPC):
                    ps = ps_agg.tile([128, sg], f32)
                    for k, (off, klen) in enumerate(kts):
                        t = g * nk + k
                        nc.tensor.matmul(
                            ps[:],
                            nodemajor[:klen, t * 128:t * 128 + 128],
                            at[:klen, t, :],
                            start=(k == 0),
                            stop=(k == nk - 1),
                        )
                    nc.vector.tensor_copy(aggT[:, g * sg:(g + 1) * sg], ps[:])
                return aggT

            def dense2(wa, rhsa, wb, rhsb, bias, func, name):
                """out[f',n] = func(wa.T@rhsa + wb.T@rhsb + bias)."""
                out = slab.tile([128, npc], cdt, name=name, tag=name)
                for c in range(nch):
                    sl = slice(c * NCHUNK, (c + 1) * NCHUNK)
                    ps = ps_d.tile([128, NCHUNK], f32)
                    nc.tensor.matmul(ps[:], wa[:], rhsa[:, sl], start=True,
                                     stop=False)
                    nc.tensor.matmul(ps[:], wb[:], rhsb[:, sl], start=False,
                                     stop=True)
                    nc.scalar.activation(out[:, sl], ps[:], func, bias=bias[:])
                return out

            def to_nodemajor(featmajor, name):
                """Per-graph k-tiles: col block g*nk+k holds nodes
                [g*sg+off, g*sg+off+klen) in partitions [0, klen)."""
                out = slab.tile([128, GPC * nk * 128], cdt, name=name, tag=name)
                for g in range(GPC):
                    for k, (off, klen) in enumerate(kts):
                        t = g * nk + k
                        ps = ps_tp.tile([128, 128], cdt)
                        nc.tensor.transpose(
                            ps[:klen, :],
                            featmajor[:, g * sg + off:g * sg + off + klen],
                            ident[:],
                        )
                        nc.vector.tensor_copy(
                            out[:klen, t * 128:t * 128 + 128], ps[:klen, :]
                        )
                return out

            xN = to_nodemajor(xT, "xN")
            agg1 = aggregate(xN, "agg1")
            h1 = dense2(W["WL1"], agg1, W["WR1"], xT, B["b1"], AF.Relu, "h1")
            h1N = to_nodemajor(h1, "h1N")
            agg2 = aggregate(h1N, "agg2")
            h2 = dense2(W["WL2"], agg2, W["WR2"], h1, B["b2"], AF.Relu, "h2")
            hout = dense2(W["WLA"], h2, W["WLB"], h1, B["b3"], AF.Identity,
                          "hout")

            uvT = slab.tile([2, npc], f32, tag="uv")
            for c in range(nch):
                sl = slice(c * NCHUNK, (c + 1) * NCHUNK)
                ps = ps_uv.tile([2, NCHUNK], f32)
                nc.tensor.matmul(ps[:], w12[:], hout[:, sl], start=True,
                                 stop=True)
                nc.vector.tensor_copy(uvT[:, sl], ps[:])
                # split output DMAs per chunk so stores overlap compute;
                # gpsimd SWDGE keeps the SP queue free for input loads
                nc.gpsimd.dma_start(hout_d[:, sl], hout[:, sl])
                nc.gpsimd.dma_start(uv_d[:, sl], uvT[:, sl])

    _split_multi_waits(nc)
    return nc


USE_BF16 = True


def _np_cdt():
    if USE_BF16:
        import ml_dtypes

        return ml_dtypes.bfloat16
    return np.float32


def _get_level_nc(sg):
    key = (sg, USE_BF16)
    if key not in _compiled:
        _compiled[key] = _build_level_nc(sg, USE_BF16)
    return _compiled[key]


def _run_level(Xslots, AThost, W, n, sg):
    """Xslots: [G, PADMAX, F] node-major; AThost: [G, sg, sg] normalized.
    W: level weight dict; n: [G] active counts (n <= sg).
    Returns Hout [G, sg, F], U [G, sg], V [G, sg] (all fp32)."""
    from concourse.bass_utils import run_bass_kernel_spmd

    nc = _get_level_nc(sg)
    kts = _ktiles(sg)
    nk = len(kts)
    npc = GPC * sg
    cdt = _np_cdt()
    wpack = np.concatenate(
        [W["Wl1"].T, W["Wr1"].T, W["Wl2"].T, W["Wr2"].T,
         W["WlinA"], W["WlinB"], np.stack([W["w1"], W["w2"]], axis=1)],
        axis=1,
    ).astype(cdt)
    bpack = np.stack(
        [W["bl1"], W["bl2"], W["blin"]], axis=1
    ).astype(np.float32)
    shared = {
        "wpack": np.ascontiguousarray(wpack),
        "bpack": np.ascontiguousarray(bpack),
    }
    in_maps = []
    for c in range(NCORES):
        xs = Xslots[c * GPC:(c + 1) * GPC, :sg].reshape(npc, F).astype(cdt)
        xT = np.ascontiguousarray(xs.T)
        at = np.zeros((128, GPC * nk, sg), cdt)
        for g in range(GPC):
            for k, (off, klen) in enumerate(kts):
                t = g * nk + k
                at[:klen, t, :] = AThost[c * GPC + g, off:off + klen, :].astype(cdt)
        in_maps.append({"xT": xT, "AT": at, **shared})

    res = run_bass_kernel_spmd(nc, in_maps, list(range(NCORES)))
    Hout = np.empty((G, sg, F), np.float32)
    U = np.empty((G, sg), np.float32)
    V = np.empty((G, sg), np.float32)
    for c in range(NCORES):
        h = res.results[c]["houtT"].astype(np.float32)   # [128, npc]
        uv = res.results[c]["uv"]                        # [2, npc]
        Hout[c * GPC:(c + 1) * GPC] = h.T.reshape(GPC, sg, F)
        U[c * GPC:(c + 1) * GPC] = uv[0].reshape(GPC, sg)
        V[c * GPC:(c + 1) * GPC] = uv[1].reshape(GPC, sg)
    return Hout, U, V


# ------------------------------------------------------------------ host ---

def _build_AT(edges, sg):
    AT = np.zeros((G, sg, sg), np.float32)
    for g in range(G):
        ls, ld = edges[g]
        if len(ls) == 0:
            continue
        cnt = np.bincount(ls.astype(np.int64) * sg + ld, minlength=sg * sg)
        cnt = cnt.reshape(sg, sg).astype(np.float32)
        indeg = np.bincount(ld, minlength=sg).astype(np.float32)
        AT[g] = cnt / np.maximum(indeg, 1.0)[None, :]
    return AT


def _pool_graph(Hout_g, u_g, v_g, ls, ld, n_g, bp):
    raw = u_g[ls] + v_g[ld] + np.float32(bp)
    m = np.full(n_g, -np.inf, np.float32)
    np.maximum.at(m, ld, raw)
    e = np.exp(raw - m[ld], dtype=np.float32)
    z = np.bincount(ld, weights=e, minlength=n_g).astype(np.float32)
    score = e / z[ld] + np.float32(0.5)

    order = np.argsort(-score, kind="stable")
    rem = [True] * n_g
    cluster = np.empty(n_g, np.int64)
    cnt = 0
    cscores = []
    ls_l = ls.tolist()
    ld_l = ld.tolist()
    sc_l = score.tolist()
    for idx in order.tolist():
        s = ls_l[idx]
        d = ld_l[idx]
        if rem[s] and rem[d]:
            cluster[s] = cnt
            cluster[d] = cnt
            rem[s] = False
            rem[d] = False
            cscores.append(sc_l[idx])
            cnt += 1
    rem_nodes = np.flatnonzero(rem)
    cluster[rem_nodes] = cnt + np.arange(len(rem_nodes))
    n_new = cnt + len(rem_nodes)

    csc = np.concatenate(
        [np.asarray(cscores, np.float32), np.ones(len(rem_nodes), np.float32)]
    )
    newX = np.zeros((PADMAX, F), np.float32)
    np.add.at(newX, cluster, Hout_g[:n_g])
    newX[:n_new] *= csc[:, None]

    keys = np.unique(cluster[ls] * PADMAX + cluster[ld])
    return newX, n_new, (
        (keys // PADMAX).astype(np.int32),
        (keys % PADMAX).astype(np.int32),
    )


def _level_weights(params, i):
    Wlin = params["Wlin%d" % i]
    Wp = params["Wp%d" % i]
    return {
        "Wl1": params["W%dl" % (2 * i - 1)],
        "bl1": params["b%dl" % (2 * i - 1)],
        "Wr1": params["W%dr" % (2 * i - 1)],
        "Wl2": params["W%dl" % (2 * i)],
        "bl2": params["b%dl" % (2 * i)],
        "Wr2": params["W%dr" % (2 * i)],
        "WlinA": np.ascontiguousarray(Wlin[:, :H].T),
        "WlinB": np.ascontiguousarray(Wlin[:, H:].T),
        "blin": params["blin%d" % i],
        "w1": np.ascontiguousarray(Wp[0, :H]),
        "w2": np.ascontiguousarray(Wp[0, H:]),
        "bp": float(np.asarray(params["bp%d" % i]).reshape(-1)[0]),
    }


def _log_softmax(x):
    m = x.max(axis=1, keepdims=True)
    e = np.exp(x - m)
    return (x - m) - np.log(e.sum(axis=1, keepdims=True))


def _pick_sg(nmax):
    for sg in (128, 256, 320):
        if nmax <= sg:
            return sg
    raise AssertionError("graph size %d exceeds 320" % nmax)


def kernel(x, edge_index, batch, params):
    x = np.asarray(x, np.float32)
    params = {k: np.asarray(v, np.float32) for k, v in params.items()}
    src = np.asarray(edge_index[0], np.int64)
    dst = np.asarray(edge_index[1], np.int64)

    g_of_edge = src // NPG
    order = np.argsort(g_of_edge, kind="stable")
    bounds = np.searchsorted(g_of_edge[order], np.arange(G + 1))
    edges = []
    for g in range(G):
        sel = order[bounds[g]: bounds[g + 1]]
        edges.append(((src[sel] - g * NPG).astype(np.int32),
                      (dst[sel] - g * NPG).astype(np.int32)))

    X = np.zeros((G, PADMAX, F), np.float32)
    X[:, :NPG] = x.reshape(G, NPG, F)
    n = np.full(G, NPG, np.int64)

    total_sum = np.zeros((G, F), np.float32)
    total_cnt = np.zeros(G, np.int64)

    for i in (1, 2, 3):
        W = _level_weights(params, i)
        sg = _pick_sg(int(n.max()))
        AT = _build_AT(edges, sg)
        Hout, U, V = _run_level(X, AT, W, n, sg)
        newX = np.zeros((G, PADMAX, F), np.float32)
        new_n = np.empty(G, np.int64)
        new_edges = []
        for g in range(G):
            total_sum[g] += Hout[g, : n[g]].sum(axis=0)
            total_cnt[g] += n[g]
            nx, nn, ne = _pool_graph(
                Hout[g], U[g], V[g], edges[g][0], edges[g][1], n[g], W["bp"]
            )
            newX[g] = nx
            new_n[g] = nn
            new_edges.append(ne)
        X, n, edges = newX, new_n, new_edges

    for g in range(G):
        total_sum[g] += X[g, : n[g]].sum(axis=0)
        total_cnt[g] += n[g]

    gv = total_sum / np.maximum(total_cnt, 1)[:, None].astype(np.float32)
    g1 = np.maximum(gv @ params["Wfc1"].T + params["bfc1"], 0.0)
    out = g1 @ params["Wfc2"].T + params["bfc2"]
    return _log_softmax(out).astype(np.float32)


# revision 21
# speedup vs baseline: 1.8650x; 1.0308x over previous
"""Trainium2 Bass kernel for nn_GCNDiehlq1 (SAGEConv x2 + lin + EdgePooling, x3
levels, mean readout + MLP + log_softmax).

Structure (validated against the jax reference at ~1e-7 rel err):
- All edges are within-graph, so SAGE mean aggregation == per-graph dense
  A_norm @ x with AT[s,d] = count(s->d)/max(indeg(d),1). The 64 graphs are
  sharded 8-per-core across 8 NeuronCores.
- The device kernel computes one level (2 SAGE convs + lin + edge-score
  projections u,v) for 8 graphs in feature-major layout. The builder is
  parameterized by slots-per-graph (320 for level 1; levels 2/3 pick
  256/128 variants based on actual post-pool graph sizes) -> one NEFF per
  size variant, reused across calls.
- Host does the inherently sequential EdgePooling (per-dst softmax from
  u,v, stable sort, greedy merge scan, coalesce) and the final readout.
  Per-graph compact cluster relabeling is strictly order-preserving
  w.r.t. the reference's global labels within each graph, so coalesce
  order and sort tie-breaks match the reference exactly.
"""

import numpy as np

N = 20480
NPG = 320
G = 64
F = 128
H = 128
C = 6
PADMAX = 384                 # host-side cluster-id padding (>= 320)
NCORES = 8
GPC = G // NCORES            # 8 graphs per core
NCHUNK = 512                 # dense matmul moving free dim

_compiled = {}


def _ktiles(sg):
    """K-tile (offset, length) list for sg slots per graph."""
    out = []
    off = 0
    while off < sg:
        out.append((off, min(128, sg - off)))
        off += 128
    return out


# ---------------------------------------------------------------- device ---

def _apply_tile_patch():
    """This walrus build rejects >1 sem waits on TPB_CTRL (Drain/NoOp):
    'Too many sync wait commands'. Split the TileContext exit-barrier waits
    across one NOP per logical proc, then emit the drain bare."""
    import concourse.tile as tile
    from concourse.vector_clock import ScopedClock, VectorClock

    if getattr(tile.TileContext, "_drain_patched", False):
        return

    def _patched(self, tick_clock, wait_clock):
        full = tick_clock.global_clock
        nprocs = len(full)
        for proc in range(nprocs):
            tick = full[proc]
            if tick <= 0:
                continue
            vec = [0] * nprocs
            vec[proc] = tick
            nop_inst = self.nc.sync.nop(nofuse=True, hint="pre_drain_%d" % proc)
            wait_clock.add_sem_waits(
                nop_inst.ins, ScopedClock({None: VectorClock(vec)})
            )
        self.nc.sync.drain()
        self.nc.all_engine_barrier()
        assert self.sems is not None
        popped = self.nc._tile_sem_poison_stack.pop()
        assert popped is self._sem_poison
        if getattr(type(self), "_keep_sem_reset", True):
            self.nc.clear_and_free_semaphores(
                list(self.sems.allocated().values()))
            self.nc.all_engine_barrier()
        else:
            # skip the end-of-NEFF sem clear + second barrier; only do the
            # python-side free-pool bookkeeping
            sem_nums = [s.num for s in self.sems.allocated().values()]
            self.nc._state.prepend_free_semaphores(sem_nums)
            for poison_set in self.nc._tile_sem_poison_stack:
                poison_set.update(sem_nums)

    tile.TileContext._drain_and_barrier = _patched
    tile.TileContext._drain_patched = True
    # re-execution works without the end-of-NEFF sem clear: the runtime
    # reinitializes semaphore state per execution (verified empirically,
    # two back-to-back kernel() calls bitwise identical)
    tile.TileContext._keep_sem_reset = False


def _split_multi_waits(nc):
    """This walrus build allows at most ONE sync wait per instruction.
    Insert single-wait NoOps (same engine, just before) for the extras."""
    import concourse.mybir as mybir

    for f in nc.m.functions:
        for bb in f.blocks:
            insts = list(bb.instructions)
            out = []
            changed = False
            for ins in insts:
                si = ins.sync_info
                if si is not None and len(si.on_wait) > 1:
                    waits = list(si.on_wait)
                    for j, w in enumerate(waits[:-1]):
                        nop = mybir.InstNoOp(name="%s_w%d" % (ins.name, j))
                        nop.engine = ins.engine
                        nop.sync_info = mybir.SyncInfo(on_wait=[w], on_update=[])
                        out.append(nop)
                    ins.sync_info = mybir.SyncInfo(
                        on_wait=[waits[-1]], on_update=list(si.on_update)
                    )
                    changed = True
                out.append(ins)
            if changed:
                bb.instructions = out


def _build_level_nc(sg, use_bf16):
    """One level for 8 graphs with sg slots per graph, feature-major
    activations [128, 8*sg]."""
    import concourse.bass as bass
    import concourse.mybir as mybir
    import concourse.tile as tile
    from concourse.masks import make_identity

    _apply_tile_patch()
    f32 = mybir.dt.float32
    cdt = mybir.dt.bfloat16 if use_bf16 else f32
    AF = mybir.ActivationFunctionType

    kts = _ktiles(sg)
    nk = len(kts)
    npc = GPC * sg
    nch = (npc + NCHUNK - 1) // NCHUNK
    assert npc % NCHUNK == 0

    nc = bass.Bass("TRN2", target_bir_lowering=False)
    xT_d = nc.declare_dram_parameter("xT", [128, npc], cdt, isOutput=False)
    at_d = nc.declare_dram_parameter("AT", [128, GPC * nk, sg], cdt,
                                     isOutput=False)
    # all six [128,128] weights + w12 packed into one [128, 770] slab
    wpack_d = nc.declare_dram_parameter("wpack", [128, 6 * 128 + 2], cdt,
                                        isOutput=False)
    bpack_d = nc.declare_dram_parameter("bpack", [128, 3], f32, isOutput=False)
    hout_d = nc.declare_dram_parameter("houtT", [128, npc], cdt, isOutput=True)
    uv_d = nc.declare_dram_parameter("uv", [2, npc], f32, isOutput=True)

    with tile.TileContext(nc) as tc:
        with (
            tc.tile_pool(name="slab", bufs=1) as slab,
            tc.tile_pool(name="wts", bufs=1) as wts,
            tc.tile_pool(name="ps_agg", bufs=2, space="PSUM") as ps_agg,
            tc.tile_pool(name="ps_d", bufs=2, space="PSUM") as ps_d,
            tc.tile_pool(name="ps_tp", bufs=2, space="PSUM") as ps_tp,
            tc.tile_pool(name="ps_uv", bufs=2, space="PSUM") as ps_uv,
        ):
            ident = wts.tile([128, 128], cdt)
            make_identity(nc, ident[:])

            # weights via gpsimd (SWDGE) to keep the SP queue free for inputs
            wpack = wts.tile([128, 6 * 128 + 2], cdt)
            nc.gpsimd.dma_start(wpack[:], wpack_d[:])
            bpack = wts.tile([128, 3], f32)
            nc.gpsimd.dma_start(bpack[:], bpack_d[:])
            wnames = ("WL1", "WR1", "WL2", "WR2", "WLA", "WLB")
            W = {nm: wpack[:, i * 128:(i + 1) * 128]
                 for i, nm in enumerate(wnames)}
            w12 = wpack[:, 6 * 128:6 * 128 + 2]
            B = {"b%d" % (i + 1): bpack[:, i:i + 1] for i in range(3)}

            # inputs on the SP queue: per-chunk xT first, then per-graph AT
            xT = slab.tile([128, npc], cdt, tag="xT")
            for c in range(nch):
                sl = slice(c * NCHUNK, (c + 1) * NCHUNK)
                nc.sync.dma_start(xT[:, sl], xT_d[:, sl])
            at = slab.tile([128, GPC * nk, sg], cdt, tag="at")
            for g in range(GPC):
                gsl = slice(g * nk, (g + 1) * nk)
                nc.sync.dma_start(at[:, gsl, :], at_d[:, gsl, :])

            def aggregate(nodemajor, name):
                """aggT[f, d] = sum_s x[s,f] * AT[s,d], per graph."""
                aggT = slab.tile([128, npc], cdt, name=name, tag=name)
                for g in range(GPC):
                    ps = ps_agg.tile([128, sg], f32)
                    for k, (off, klen) in enumerate(kts):
                        t = g * nk + k
                        nc.tensor.matmul(
                            ps[:],
                            nodemajor[:klen, t * 128:t * 128 + 128],
                            at[:klen, t, :],
                            start=(k == 0),
                            stop=(k == nk - 1),
                        )
                    nc.vector.tensor_copy(aggT[:, g * sg:(g + 1) * sg], ps[:])
                return aggT

            def dense2(wa, rhsa, wb, rhsb, bias, func, name):
                """out[f',n] = func(wa.T@rhsa + wb.T@rhsb + bias)."""
                out = slab.tile([128, npc], cdt, name=name, tag=name)
                for c in range(nch):
                    sl = slice(c * NCHUNK, (c + 1) * NCHUNK)
                    ps = ps_d.tile([128, NCHUNK], f32)
                    nc.tensor.matmul(ps[:], wa[:], rhsa[:, sl], start=True,
                                     stop=False)
                    nc.tensor.matmul(ps[:], wb[:], rhsb[:, sl], start=False,
                                     stop=True)
                    nc.scalar.activation(out[:, sl], ps[:], func, bias=bias[:])
                return out

            def to_nodemajor(featmajor, name):
                """Per-graph k-tiles: col block g*nk+k holds nodes
                [g*sg+off, g*sg+off+klen) in partitions [0, klen)."""
                out = slab.tile([128, GPC * nk * 128], cdt, name=name, tag=name)
                for g in range(GPC):
                    for k, (off, klen) in enumerate(kts):
                        t = g * nk + k
                        ps = ps_tp.tile([128, 128], cdt)
                        nc.tensor.transpose(
                            ps[:klen, :],
                            featmajor[:, g * sg + off:g * sg + off + klen],
                            ident[:],
                        )
                        nc.vector.tensor_copy(
                            out[:klen, t * 128:t * 128 + 128], ps[:klen, :]
                        )
                return out

            xN = to_nodemajor(xT, "xN")
            agg1 = aggregate(xN, "agg1")
            h1 = dense2(W["WL1"], agg1, W["WR1"], xT, B["b1"], AF.Relu, "h1")
            h1N = to_nodemajor(h1, "h1N")
            agg2 = aggregate(h1N, "agg2")
            h2 = dense2(W["WL2"], agg2, W["WR2"], h1, B["b2"], AF.Relu, "h2")
            hout = dense2(W["WLA"], h2, W["WLB"], h1, B["b3"], AF.Identity,
                          "hout")

            uvT = slab.tile([2, npc], f32, tag="uv")
            for c in range(nch):
                sl = slice(c * NCHUNK, (c + 1) * NCHUNK)
                ps = ps_uv.tile([2, NCHUNK], f32)
                nc.tensor.matmul(ps[:], w12[:], hout[:, sl], start=True,
                                 stop=True)
                nc.vector.tensor_copy(uvT[:, sl], ps[:])
                # split output DMAs per chunk so stores overlap compute;
                # gpsimd SWDGE keeps the SP queue free for input loads
                nc.gpsimd.dma_start(hout_d[:, sl], hout[:, sl])
                nc.gpsimd.dma_start(uv_d[:, sl], uvT[:, sl])

    _split_multi_waits(nc)
    return nc


USE_BF16 = True


def _np_cdt():
    if USE_BF16:
        import ml_dtypes

        return ml_dtypes.bfloat16
    return np.float32


def _get_level_nc(sg):
    key = (sg, USE_BF16)
    if key not in _compiled:
        _compiled[key] = _build_level_nc(sg, USE_BF16)
    return _compiled[key]


def _run_level(Xslots, AThost, W, n, sg):
    """Xslots: [G, PADMAX, F] node-major; AThost: [G, sg, sg] normalized.
    W: level weight dict; n: [G] active counts (n <= sg).
    Returns Hout [G, sg, F], U [G, sg], V [G, sg] (all fp32)."""
    from concourse.bass_utils import run_bass_kernel_spmd

    nc = _get_level_nc(sg)
    kts = _ktiles(sg)
    nk = len(kts)
    npc = GPC * sg
    cdt = _np_cdt()
    wpack = np.concatenate(
        [W["Wl1"].T, W["Wr1"].T, W["Wl2"].T, W["Wr2"].T,
         W["WlinA"], W["WlinB"], np.stack([W["w1"], W["w2"]], axis=1)],
        axis=1,
    ).astype(cdt)
    bpack = np.stack(
        [W["bl1"], W["bl2"], W["blin"]], axis=1
    ).astype(np.float32)
    shared = {
        "wpack": np.ascontiguousarray(wpack),
        "bpack": np.ascontiguousarray(bpack),
    }
    in_maps = []
    for c in range(NCORES):
        xs = Xslots[c * GPC:(c + 1) * GPC, :sg].reshape(npc, F).astype(cdt)
        xT = np.ascontiguousarray(xs.T)
        at = np.zeros((128, GPC * nk, sg), cdt)
        for g in range(GPC):
            for k, (off, klen) in enumerate(kts):
                t = g * nk + k
                at[:klen, t, :] = AThost[c * GPC + g, off:off + klen, :].astype(cdt)
        in_maps.append({"xT": xT, "AT": at, **shared})

    res = run_bass_kernel_spmd(nc, in_maps, list(range(NCORES)))
    Hout = np.empty((G, sg, F), np.float32)
    U = np.empty((G, sg), np.float32)
    V = np.empty((G, sg), np.float32)
    for c in range(NCORES):
        h = res.results[c]["houtT"].astype(np.float32)   # [128, npc]
        uv = res.results[c]["uv"]                        # [2, npc]
        Hout[c * GPC:(c + 1) * GPC] = h.T.reshape(GPC, sg, F)
        U[c * GPC:(c + 1) * GPC] = uv[0].reshape(GPC, sg)
        V[c * GPC:(c + 1) * GPC] = uv[1].reshape(GPC, sg)
    return Hout, U, V


# ------------------------------------------------------------------ host ---

def _build_AT(edges, sg):
    AT = np.zeros((G, sg, sg), np.float32)
    for g in range(G):
        ls, ld = edges[g]
        if len(ls) == 0:
            continue
        cnt = np.bincount(ls.astype(np.int64) * sg + ld, minlength=sg * sg)
        cnt = cnt.reshape(sg, sg).astype(np.float32)
        indeg = np.bincount(ld, minlength=sg).astype(np.float32)
        AT[g] = cnt / np.maximum(indeg, 1.0)[None, :]
    return AT


def _pool_graph(Hout_g, u_g, v_g, ls, ld, n_g, bp):
    raw = u_g[ls] + v_g[ld] + np.float32(bp)
    m = np.full(n_g, -np.inf, np.float32)
    np.maximum.at(m, ld, raw)
    e = np.exp(raw - m[ld], dtype=np.float32)
    z = np.bincount(ld, weights=e, minlength=n_g).astype(np.float32)
    score = e / z[ld] + np.float32(0.5)

    order = np.argsort(-score, kind="stable")
    rem = [True] * n_g
    cluster = np.empty(n_g, np.int64)
    cnt = 0
    cscores = []
    ls_l = ls.tolist()
    ld_l = ld.tolist()
    sc_l = score.tolist()
    for idx in order.tolist():
        s = ls_l[idx]
        d = ld_l[idx]
        if rem[s] and rem[d]:
            cluster[s] = cnt
            cluster[d] = cnt
            rem[s] = False
            rem[d] = False
            cscores.append(sc_l[idx])
            cnt += 1
    rem_nodes = np.flatnonzero(rem)
    cluster[rem_nodes] = cnt + np.arange(len(rem_nodes))
    n_new = cnt + len(rem_nodes)

    csc = np.concatenate(
        [np.asarray(cscores, np.float32), np.ones(len(rem_nodes), np.float32)]
    )
    newX = np.zeros((PADMAX, F), np.float32)
    np.add.at(newX, cluster, Hout_g[:n_g])
    newX[:n_new] *= csc[:, None]

    keys = np.unique(cluster[ls] * PADMAX + cluster[ld])
    return newX, n_new, (
        (keys // PADMAX).astype(np.int32),
        (keys % PADMAX).astype(np.int32),
    )


def _level_weights(params, i):
    Wlin = params["Wlin%d" % i]
    Wp = params["Wp%d" % i]
    return {
        "Wl1": params["W%dl" % (2 * i - 1)],
        "bl1": params["b%dl" % (2 * i - 1)],
        "Wr1": params["W%dr" % (2 * i - 1)],
        "Wl2": params["W%dl" % (2 * i)],
        "bl2": params["b%dl" % (2 * i)],
        "Wr2": params["W%dr" % (2 * i)],
        "WlinA": np.ascontiguousarray(Wlin[:, :H].T),
        "WlinB": np.ascontiguousarray(Wlin[:, H:].T),
        "blin": params["blin%d" % i],
        "w1": np.ascontiguousarray(Wp[0, :H]),
        "w2": np.ascontiguousarray(Wp[0, H:]),
        "bp": float(np.asarray(params["bp%d" % i]).reshape(-1)[0]),
    }


def _log_softmax(x):
    m = x.max(axis=1, keepdims=True)
    e = np.exp(x - m)
    return (x - m) - np.log(e.sum(axis=1, keepdims=True))


def _pick_sg(nmax):
    for sg in (128, 256, 320):
        if nmax <= sg:
            return sg
    raise AssertionError("graph size %d exceeds 320" % nmax)


def kernel(x, edge_index, batch, params):
    x = np.asarray(x, np.float32)
    params = {k: np.asarray(v, np.float32) for k, v in params.items()}
    src = np.asarray(edge_index[0], np.int64)
    dst = np.asarray(edge_index[1], np.int64)

    g_of_edge = src // NPG
    order = np.argsort(g_of_edge, kind="stable")
    bounds = np.searchsorted(g_of_edge[order], np.arange(G + 1))
    edges = []
    for g in range(G):
        sel = order[bounds[g]: bounds[g + 1]]
        edges.append(((src[sel] - g * NPG).astype(np.int32),
                      (dst[sel] - g * NPG).astype(np.int32)))

    X = np.zeros((G, PADMAX, F), np.float32)
    X[:, :NPG] = x.reshape(G, NPG, F)
    n = np.full(G, NPG, np.int64)

    total_sum = np.zeros((G, F), np.float32)
    total_cnt = np.zeros(G, np.int64)

    for i in (1, 2, 3):
        W = _level_weights(params, i)
        sg = _pick_sg(int(n.max()))
        AT = _build_AT(edges, sg)
        Hout, U, V = _run_level(X, AT, W, n, sg)
        newX = np.zeros((G, PADMAX, F), np.float32)
        new_n = np.empty(G, np.int64)
        new_edges = []
        for g in range(G):
            total_sum[g] += Hout[g, : n[g]].sum(axis=0)
            total_cnt[g] += n[g]
            nx, nn, ne = _pool_graph(
                Hout[g], U[g], V[g], edges[g][0], edges[g][1], n[g], W["bp"]
            )
            newX[g] = nx
            new_n[g] = nn
            new_edges.append(ne)
        X, n, edges = newX, new_n, new_edges

    for g in range(G):
        total_sum[g] += X[g, : n[g]].sum(axis=0)
        total_cnt[g] += n[g]

    gv = total_sum / np.maximum(total_cnt, 1)[:, None].astype(np.float32)
    g1 = np.maximum(gv @ params["Wfc1"].T + params["bfc1"], 0.0)
    out = g1 @ params["Wfc2"].T + params["bfc2"]
    return _log_softmax(out).astype(np.float32)
